# revision 4
# baseline (speedup 1.0000x reference)
"""Conv2D 3x3 (stride 1, pad 1) NCHW on 8 TRN2 NeuronCores.

x: (32, 128, 56, 56) f32, weight: (256, 128, 3, 3) OIHW, bias: (256,)
out: (32, 256, 56, 56) f32.

Strategy: data-parallel over batch (4 images per core, weight/bias
replicated). The input is zero-padded to 58x58 and converted to bf16 on the
host (rel err ~3e-3 << 2e-2 gate; PSUM accumulates in f32). Each padded
image lives in SBUF with C_in=128 on partitions; the 3x3 conv is 9 shifted
[128x128] @ [128x448] bf16 matmuls accumulated in PSUM (output tile = 8
rows x 56 cols per co-tile). bf16 enables the compiler's Fast Weight Load,
so the per-matmul LDWEIGHTS (~96ns) hides behind the 187ns matmul stream.

DMA plan: the two HWDGE rings are independent FIFOs — inputs ride the
scalar ring (weights tap0 first, then the rest; image 0 in 3 chunks with
the first 10 rows leading so the first matmul can start ~1.5us in; later
images prefetch as 2 big chunks each), outputs ride the sync ring (one DMA
per [128co x 448pix] tile) so the post-compute drain tail is just the last
tile's bias-add + store.
"""

import numpy as np
import ml_dtypes

import concourse.tile as tile
from concourse import bacc, mybir
from concourse.bass_utils import run_bass_kernel_spmd

N_CORES = 8
N_BATCH = 32
N_PER_CORE = N_BATCH // N_CORES  # 4
C_IN, C_OUT, H, W = 128, 256, 56, 56
HP, WP = H + 2, W + 2  # 58 (zero-padded on host)
ROWS = 8  # output rows per PSUM tile
N_RTILES = H // ROWS  # 7
NFREE = ROWS * W  # 448 <= 512 (one PSUM bank of f32)
N_CT = C_OUT // 128  # 2 co-tiles


def build_nc(n_imgs=N_PER_CORE):
    f32 = mybir.dt.float32
    bf16 = mybir.dt.bfloat16
    nc = bacc.Bacc("TRN2", target_bir_lowering=False, debug=False)
    x = nc.dram_tensor("x", [n_imgs, C_IN, HP, WP], bf16, kind="ExternalInput")
    w = nc.dram_tensor("w", [C_IN, 9 * C_OUT], bf16, kind="ExternalInput")
    b = nc.dram_tensor("b", [C_IN, N_CT], f32, kind="ExternalInput")
    out = nc.dram_tensor("out", [n_imgs, C_OUT, H * W], f32, kind="ExternalOutput")

    with tile.TileContext(nc) as tc:
        with tc.tile_pool(name="wpool", bufs=1) as wpool, \
             tc.tile_pool(name="xpool", bufs=3) as xpool, \
             tc.tile_pool(name="opool", bufs=8) as opool, \
             tc.tile_pool(name="pspool", bufs=4, space="PSUM") as pspool, \
             tc.tile_pool(name="pdummy", bufs=1, space="PSUM") as pdummy:
            w_sb = wpool.tile([C_IN, 9 * C_OUT], bf16)
            b_sb = wpool.tile([C_IN, N_CT], f32)
            xp0 = xpool.tile([C_IN, HP, WP], bf16, tag="xp", name="xp")
            # Startup critical path: the first row-tile's matmuls need only
            # weight taps (kh,0..2) and input rows 0-9. Lead each ring with
            # exactly that, everything else follows.
            nc.sync.dma_start(xp0[:, 0:10, :], x[0, :, 0:10, :])
            nc.scalar.dma_start(w_sb[:, 0:3 * C_OUT], w[:, 0:3 * C_OUT])
            nc.sync.dma_start(xp0[:, 10:34, :], x[0, :, 10:34, :])
            nc.scalar.dma_start(w_sb[:, 3 * C_OUT:], w[:, 3 * C_OUT:])
            nc.scalar.dma_start(b_sb[:], b[:])
            nc.sync.dma_start(xp0[:, 34:58, :], x[0, :, 34:58, :])

            # HAM warm-up: the PE clock sits at 1.2 GHz until ~3.4us of
            # sustained matmul activity. The first real matmul can't start
            # until its DMAs land (~10.6us: ~7.2us framework preamble +
            # ~3.4us cold DMA completion), so burn that wait on dummy
            # matmuls over memset tiles — real matmuls then start at the
            # full 2.4 GHz instead of paying ~2us of half-clock ramp.
            dw = wpool.tile([128, 128], bf16)
            dx = wpool.tile([128, NFREE], bf16)
            nc.gpsimd.memset(dw[:], 0.0)
            nc.gpsimd.memset(dx[:], 0.0)
            dpt = pdummy.tile([128, NFREE], f32)
            for _ in range(8):
                nc.tensor.matmul(dpt[:], dw[:], dx[:], start=True, stop=True)

            for n in range(n_imgs):
                if n == 0:
                    xp = xp0
                else:
                    xp = xpool.tile([C_IN, HP, WP], bf16, tag="xp", name="xp")
                    nc.scalar.dma_start(xp[:, 0:29, :], x[n, :, 0:29, :])
                    nc.scalar.dma_start(xp[:, 29:58, :], x[n, :, 29:58, :])
                for r in range(N_RTILES):
                    for ct in range(N_CT):
                        pt = pspool.tile([128, NFREE], f32, tag="pt")
                        for tap in range(9):
                            kh, kw = tap // 3, tap % 3
                            c0 = tap * C_OUT + ct * 128
                            nc.tensor.matmul(
                                pt[:],
                                w_sb[:, c0:c0 + 128],
                                xp[:, r * ROWS + kh:r * ROWS + kh + ROWS, kw:kw + W],
                                start=(tap == 0),
                                stop=(tap == 8),
                            )
                        last = n == n_imgs - 1 and r == N_RTILES - 1 and ct == N_CT - 1
                        if not last:
                            ot = opool.tile([128, NFREE], f32, tag="ot")
                            nc.vector.tensor_scalar_add(ot[:], pt[:],
                                                        b_sb[:, ct:ct + 1])
                            nc.sync.dma_start(
                                out[n, ct * 128:(ct + 1) * 128,
                                    r * NFREE:(r + 1) * NFREE],
                                ot[:],
                            )
                        else:
                            # last tile: bias-add halves on vector + scalar
                            # in parallel, two stores, to shorten the drain
                            # tail after the final matmul.
                            half = NFREE // 2
                            ot0 = opool.tile([128, half], f32, tag="ot")
                            ot1 = opool.tile([128, half], f32, tag="ot")
                            nc.vector.tensor_scalar_add(ot0[:], pt[:, 0:half],
                                                        b_sb[:, ct:ct + 1])
                            nc.scalar.activation(
                                ot1[:], pt[:, half:NFREE],
                                func=mybir.ActivationFunctionType.Identity,
                                bias=b_sb[:, ct:ct + 1])
                            nc.sync.dma_start(
                                out[n, ct * 128:(ct + 1) * 128,
                                    r * NFREE:r * NFREE + half],
                                ot0[:],
                            )
                            nc.sync.dma_start(
                                out[n, ct * 128:(ct + 1) * 128,
                                    r * NFREE + half:(r + 1) * NFREE],
                                ot1[:],
                            )
    nc.compile()
    return nc


def _host_prep(x, weight, bias):
    # zero-pad H and W by 1 on the host, convert to bf16 (RTNE)
    xp = np.pad(np.asarray(x, dtype=np.float32),
                ((0, 0), (0, 0), (1, 1), (1, 1)))
    xp = np.ascontiguousarray(xp.astype(ml_dtypes.bfloat16))
    # weight OIHW -> [ci, (kh kw co)] so each lhsT tile is a contiguous slice
    w_host = np.ascontiguousarray(
        np.asarray(weight, dtype=np.float32)
        .transpose(1, 2, 3, 0).reshape(C_IN, 9 * C_OUT)
        .astype(ml_dtypes.bfloat16)
    )
    # bias[co] -> [co % 128, co // 128]
    b_host = np.ascontiguousarray(
        np.asarray(bias, dtype=np.float32).reshape(N_CT, 128).T)
    return xp, w_host, b_host


def kernel(x, weight, bias, _trace=False):
    xp, w_host, b_host = _host_prep(x, weight, bias)
    nc = build_nc()
    in_maps = [
        {"x": xp[i * N_PER_CORE:(i + 1) * N_PER_CORE], "w": w_host, "b": b_host}
        for i in range(N_CORES)
    ]
    res = run_bass_kernel_spmd(nc, in_maps, core_ids=list(range(N_CORES)), trace=_trace)
    out = np.concatenate(
        [res.results[i]["out"].reshape(N_PER_CORE, C_OUT, H, W) for i in range(N_CORES)],
        axis=0,
    )
    if _trace:
        return out, res
    return out


# revision 5
# speedup vs baseline: 1.0166x; 1.0166x over previous
"""Conv2D 3x3 (stride 1, pad 1) NCHW on 8 TRN2 NeuronCores.

x: (32, 128, 56, 56) f32, weight: (256, 128, 3, 3) OIHW, bias: (256,)
out: (32, 256, 56, 56) f32.

Strategy: data-parallel over batch (4 images per core, weight/bias
replicated). The input is zero-padded to 58x58 and converted to bf16 on the
host (rel err ~3e-3 << 2e-2 gate; PSUM accumulates in f32). Each padded
image lives in SBUF with C_in=128 on partitions; the 3x3 conv is 9 shifted
[128x128] @ [128x448] bf16 matmuls accumulated in PSUM (output tile = 8
rows x 56 cols per co-tile). bf16 enables the compiler's Fast Weight Load,
so the per-matmul LDWEIGHTS (~96ns) hides behind the 187ns matmul stream.

DMA plan: the two HWDGE rings are independent FIFOs — inputs ride the
scalar ring (weights tap0 first, then the rest; image 0 in 3 chunks with
the first 10 rows leading so the first matmul can start ~1.5us in; later
images prefetch as 2 big chunks each), outputs ride the sync ring (one DMA
per [128co x 448pix] tile) so the post-compute drain tail is just the last
tile's bias-add + store.
"""

import numpy as np
import ml_dtypes

import concourse.tile as tile
from concourse import bacc, mybir
from concourse.bass_utils import run_bass_kernel_spmd

N_CORES = 8
N_BATCH = 32
N_PER_CORE = N_BATCH // N_CORES  # 4
C_IN, C_OUT, H, W = 128, 256, 56, 56
HP, WP = H + 2, W + 2  # 58 (zero-padded on host)
ROWS = 8  # output rows per PSUM tile
N_RTILES = H // ROWS  # 7
NFREE = ROWS * W  # 448 <= 512 (one PSUM bank of f32)
N_CT = C_OUT // 128  # 2 co-tiles


def build_nc(n_imgs=N_PER_CORE):
    f32 = mybir.dt.float32
    bf16 = mybir.dt.bfloat16
    nc = bacc.Bacc("TRN2", target_bir_lowering=False, debug=False)
    x = nc.dram_tensor("x", [n_imgs, C_IN, HP, WP], bf16, kind="ExternalInput")
    w = nc.dram_tensor("w", [C_IN, 9 * C_OUT], bf16, kind="ExternalInput")
    b = nc.dram_tensor("b", [C_IN, N_CT], f32, kind="ExternalInput")
    out = nc.dram_tensor("out", [n_imgs, C_OUT, H * W], f32, kind="ExternalOutput")

    with tile.TileContext(nc) as tc:
        with tc.tile_pool(name="wpool", bufs=1) as wpool, \
             tc.tile_pool(name="xpool", bufs=3) as xpool, \
             tc.tile_pool(name="opool", bufs=8) as opool, \
             tc.tile_pool(name="pspool", bufs=4, space="PSUM") as pspool, \
             tc.tile_pool(name="pdummy", bufs=1, space="PSUM") as pdummy:
            w_sb = wpool.tile([C_IN, 9 * C_OUT], bf16)
            b_sb = wpool.tile([C_IN, N_CT], f32)
            xp0 = xpool.tile([C_IN, HP, WP], bf16, tag="xp", name="xp")
            # Startup critical path: the first row-tile's matmuls need only
            # weight taps (kh,0..2) and input rows 0-9. Lead each ring with
            # exactly that, everything else follows.
            # All nine weight taps ride ahead of the bulk x chunks, split
            # across both rings: a late tap stalls the 9-tap accumulation
            # AND resets the HAM warm-up window (measured 3.3us + re-ramp).
            nc.sync.dma_start(xp0[:, 0:10, :], x[0, :, 0:10, :])
            nc.scalar.dma_start(w_sb[:, 0:3 * C_OUT], w[:, 0:3 * C_OUT])
            nc.sync.dma_start(w_sb[:, 6 * C_OUT:], w[:, 6 * C_OUT:])
            nc.scalar.dma_start(w_sb[:, 3 * C_OUT:6 * C_OUT], w[:, 3 * C_OUT:6 * C_OUT])
            nc.scalar.dma_start(b_sb[:], b[:])
            nc.sync.dma_start(xp0[:, 10:34, :], x[0, :, 10:34, :])
            nc.sync.dma_start(xp0[:, 34:58, :], x[0, :, 34:58, :])

            # HAM warm-up: the PE clock sits at 1.2 GHz until ~3.4us of
            # sustained matmul activity. The first real matmul can't start
            # until its DMAs land (~10.6us: ~7.2us framework preamble +
            # ~3.4us cold DMA completion), so burn that wait on dummy
            # matmuls over memset tiles — real matmuls then start at the
            # full 2.4 GHz instead of paying ~2us of half-clock ramp.
            dw = wpool.tile([128, 128], bf16)
            dx = wpool.tile([128, NFREE], bf16)
            nc.gpsimd.memset(dw[:], 0.0)
            nc.gpsimd.memset(dx[:], 0.0)
            dpt = pdummy.tile([128, NFREE], f32)
            for _ in range(8):
                nc.tensor.matmul(dpt[:], dw[:], dx[:], start=True, stop=True)

            for n in range(n_imgs):
                if n == 0:
                    xp = xp0
                else:
                    xp = xpool.tile([C_IN, HP, WP], bf16, tag="xp", name="xp")
                    nc.scalar.dma_start(xp[:, 0:29, :], x[n, :, 0:29, :])
                    nc.scalar.dma_start(xp[:, 29:58, :], x[n, :, 29:58, :])
                for r in range(N_RTILES):
                    for ct in range(N_CT):
                        pt = pspool.tile([128, NFREE], f32, tag="pt")
                        for tap in range(9):
                            kh, kw = tap // 3, tap % 3
                            c0 = tap * C_OUT + ct * 128
                            nc.tensor.matmul(
                                pt[:],
                                w_sb[:, c0:c0 + 128],
                                xp[:, r * ROWS + kh:r * ROWS + kh + ROWS, kw:kw + W],
                                start=(tap == 0),
                                stop=(tap == 8),
                            )
                        last = n == n_imgs - 1 and r == N_RTILES - 1 and ct == N_CT - 1
                        if not last:
                            ot = opool.tile([128, NFREE], f32, tag="ot")
                            nc.vector.tensor_scalar_add(ot[:], pt[:],
                                                        b_sb[:, ct:ct + 1])
                            nc.sync.dma_start(
                                out[n, ct * 128:(ct + 1) * 128,
                                    r * NFREE:(r + 1) * NFREE],
                                ot[:],
                            )
                        else:
                            # last tile: bias-add halves on vector + scalar
                            # in parallel, two stores, to shorten the drain
                            # tail after the final matmul.
                            half = NFREE // 2
                            ot0 = opool.tile([128, half], f32, tag="ot")
                            ot1 = opool.tile([128, half], f32, tag="ot")
                            nc.vector.tensor_scalar_add(ot0[:], pt[:, 0:half],
                                                        b_sb[:, ct:ct + 1])
                            nc.scalar.activation(
                                ot1[:], pt[:, half:NFREE],
                                func=mybir.ActivationFunctionType.Identity,
                                bias=b_sb[:, ct:ct + 1])
                            nc.sync.dma_start(
                                out[n, ct * 128:(ct + 1) * 128,
                                    r * NFREE:r * NFREE + half],
                                ot0[:],
                            )
                            nc.sync.dma_start(
                                out[n, ct * 128:(ct + 1) * 128,
                                    r * NFREE + half:(r + 1) * NFREE],
                                ot1[:],
                            )
    nc.compile()
    return nc


def _host_prep(x, weight, bias):
    # zero-pad H and W by 1 on the host, convert to bf16 (RTNE)
    xp = np.pad(np.asarray(x, dtype=np.float32),
                ((0, 0), (0, 0), (1, 1), (1, 1)))
    xp = np.ascontiguousarray(xp.astype(ml_dtypes.bfloat16))
    # weight OIHW -> [ci, (kh kw co)] so each lhsT tile is a contiguous slice
    w_host = np.ascontiguousarray(
        np.asarray(weight, dtype=np.float32)
        .transpose(1, 2, 3, 0).reshape(C_IN, 9 * C_OUT)
        .astype(ml_dtypes.bfloat16)
    )
    # bias[co] -> [co % 128, co // 128]
    b_host = np.ascontiguousarray(
        np.asarray(bias, dtype=np.float32).reshape(N_CT, 128).T)
    return xp, w_host, b_host


def kernel(x, weight, bias, _trace=False):
    xp, w_host, b_host = _host_prep(x, weight, bias)
    nc = build_nc()
    in_maps = [
        {"x": xp[i * N_PER_CORE:(i + 1) * N_PER_CORE], "w": w_host, "b": b_host}
        for i in range(N_CORES)
    ]
    res = run_bass_kernel_spmd(nc, in_maps, core_ids=list(range(N_CORES)), trace=_trace)
    out = np.concatenate(
        [res.results[i]["out"].reshape(N_PER_CORE, C_OUT, H, W) for i in range(N_CORES)],
        axis=0,
    )
    if _trace:
        return out, res
    return out


# revision 6
# speedup vs baseline: 1.0213x; 1.0046x over previous
"""Conv2D 3x3 (stride 1, pad 1) NCHW on 8 TRN2 NeuronCores.

x: (32, 128, 56, 56) f32, weight: (256, 128, 3, 3) OIHW, bias: (256,)
out: (32, 256, 56, 56) f32.

Strategy: data-parallel over batch (4 images per core, weight/bias
replicated). The input is zero-padded to 58x58 and converted to bf16 on the
host (rel err ~3e-3 << 2e-2 gate; PSUM accumulates in f32). Each padded
image lives in SBUF with C_in=128 on partitions; the 3x3 conv is 9 shifted
[128x128] @ [128x448] bf16 matmuls accumulated in PSUM (output tile = 8
rows x 56 cols per co-tile). bf16 enables the compiler's Fast Weight Load,
so the per-matmul LDWEIGHTS (~96ns) hides behind the 187ns matmul stream.

DMA plan: the two HWDGE rings are independent FIFOs — inputs ride the
scalar ring (weights tap0 first, then the rest; image 0 in 3 chunks with
the first 10 rows leading so the first matmul can start ~1.5us in; later
images prefetch as 2 big chunks each), outputs ride the sync ring (one DMA
per [128co x 448pix] tile) so the post-compute drain tail is just the last
tile's bias-add + store.
"""

import numpy as np
import ml_dtypes

import concourse.tile as tile
from concourse import bacc, mybir
from concourse.bass_utils import run_bass_kernel_spmd

N_CORES = 8
N_BATCH = 32
N_PER_CORE = N_BATCH // N_CORES  # 4
C_IN, C_OUT, H, W = 128, 256, 56, 56
HP, WP = H + 2, W + 2  # 58 (zero-padded on host)
ROWS = 8  # output rows per PSUM tile
N_RTILES = H // ROWS  # 7
NFREE = ROWS * W  # 448 <= 512 (one PSUM bank of f32)
N_CT = C_OUT // 128  # 2 co-tiles


def build_nc(n_imgs=N_PER_CORE):
    f32 = mybir.dt.float32
    bf16 = mybir.dt.bfloat16
    nc = bacc.Bacc("TRN2", target_bir_lowering=False, debug=False)
    x = nc.dram_tensor("x", [n_imgs, C_IN, HP, WP], bf16, kind="ExternalInput")
    w = nc.dram_tensor("w", [C_IN, 9 * C_OUT], bf16, kind="ExternalInput")
    b = nc.dram_tensor("b", [C_IN, N_CT], f32, kind="ExternalInput")
    out = nc.dram_tensor("out", [n_imgs, C_OUT, H * W], f32, kind="ExternalOutput")

    with tile.TileContext(nc) as tc:
        with tc.tile_pool(name="wpool", bufs=1) as wpool, \
             tc.tile_pool(name="xpool", bufs=3) as xpool, \
             tc.tile_pool(name="opool", bufs=8) as opool, \
             tc.tile_pool(name="pspool", bufs=4, space="PSUM") as pspool, \
             tc.tile_pool(name="pdummy", bufs=1, space="PSUM") as pdummy:
            w_sb = wpool.tile([C_IN, 9 * C_OUT], bf16)
            b_sb = wpool.tile([C_IN, N_CT], f32)
            xp0 = xpool.tile([C_IN, HP, WP], bf16, tag="xp", name="xp")
            # Startup critical path: the first row-tile's matmuls need only
            # weight taps (kh,0..2) and input rows 0-9. Lead each ring with
            # exactly that, everything else follows.
            # DMA schedule. Rings are strict FIFO internally and share HBM
            # ~50/50 at packet granularity, and a DMA's semaphore fires only
            # once ALL 16 SDMA engines pass it — so order each ring so bytes
            # drain in compute-need order, weights ahead of bulk x. A late
            # weight tap stalls the 9-tap accumulation AND resets the HAM
            # warm-up window (measured: 3.3us stall + 2us re-ramp).
            nc.sync.dma_start(xp0[:, 0:10, :], x[0, :, 0:10, :])
            nc.scalar.dma_start(w_sb[:, 0:3 * C_OUT], w[:, 0:3 * C_OUT])
            nc.sync.dma_start(w_sb[:, 3 * C_OUT:6 * C_OUT], w[:, 3 * C_OUT:6 * C_OUT])
            nc.scalar.dma_start(w_sb[:, 6 * C_OUT:], w[:, 6 * C_OUT:])
            nc.scalar.dma_start(b_sb[:], b[:])
            nc.sync.dma_start(xp0[:, 10:34, :], x[0, :, 10:34, :])
            nc.sync.dma_start(xp0[:, 34:58, :], x[0, :, 34:58, :])

            # HAM warm-up: the PE clock sits at 1.2 GHz until ~3.4us of
            # sustained matmul activity. The first real matmul can't start
            # until its DMAs land (~10.6us: ~7.2us framework preamble +
            # ~3.4us cold DMA completion), so burn that wait on dummy
            # matmuls over memset tiles — real matmuls then start at the
            # full 2.4 GHz instead of paying ~2us of half-clock ramp.
            dw = wpool.tile([128, 128], bf16)
            dx = wpool.tile([128, NFREE], bf16)
            nc.gpsimd.memset(dw[:], 0.0)
            nc.gpsimd.memset(dx[:], 0.0)
            dpt = pdummy.tile([128, NFREE], f32)
            for _ in range(8):
                nc.tensor.matmul(dpt[:], dw[:], dx[:], start=True, stop=True)

            for n in range(n_imgs):
                if n == 0:
                    xp = xp0
                else:
                    xp = xpool.tile([C_IN, HP, WP], bf16, tag="xp", name="xp")
                    nc.scalar.dma_start(xp[:, 0:29, :], x[n, :, 0:29, :])
                    nc.scalar.dma_start(xp[:, 29:58, :], x[n, :, 29:58, :])
                for r in range(N_RTILES):
                    for ct in range(N_CT):
                        pt = pspool.tile([128, NFREE], f32, tag="pt")
                        for tap in range(9):
                            kh, kw = tap // 3, tap % 3
                            c0 = tap * C_OUT + ct * 128
                            nc.tensor.matmul(
                                pt[:],
                                w_sb[:, c0:c0 + 128],
                                xp[:, r * ROWS + kh:r * ROWS + kh + ROWS, kw:kw + W],
                                start=(tap == 0),
                                stop=(tap == 8),
                            )
                        last = n == n_imgs - 1 and r == N_RTILES - 1 and ct == N_CT - 1
                        if not last:
                            ot = opool.tile([128, NFREE], f32, tag="ot")
                            nc.vector.tensor_scalar_add(ot[:], pt[:],
                                                        b_sb[:, ct:ct + 1])
                            nc.sync.dma_start(
                                out[n, ct * 128:(ct + 1) * 128,
                                    r * NFREE:(r + 1) * NFREE],
                                ot[:],
                            )
                        else:
                            # last tile: bias-add halves on vector + scalar
                            # in parallel, two stores, to shorten the drain
                            # tail after the final matmul.
                            half = NFREE // 2
                            ot0 = opool.tile([128, half], f32, tag="ot")
                            ot1 = opool.tile([128, half], f32, tag="ot")
                            nc.vector.tensor_scalar_add(ot0[:], pt[:, 0:half],
                                                        b_sb[:, ct:ct + 1])
                            nc.scalar.activation(
                                ot1[:], pt[:, half:NFREE],
                                func=mybir.ActivationFunctionType.Identity,
                                bias=b_sb[:, ct:ct + 1])
                            nc.sync.dma_start(
                                out[n, ct * 128:(ct + 1) * 128,
                                    r * NFREE:r * NFREE + half],
                                ot0[:],
                            )
                            nc.sync.dma_start(
                                out[n, ct * 128:(ct + 1) * 128,
                                    r * NFREE + half:(r + 1) * NFREE],
                                ot1[:],
                            )
    nc.compile()
    return nc


def _host_prep(x, weight, bias):
    # zero-pad H and W by 1 on the host, convert to bf16 (RTNE)
    xp = np.pad(np.asarray(x, dtype=np.float32),
                ((0, 0), (0, 0), (1, 1), (1, 1)))
    xp = np.ascontiguousarray(xp.astype(ml_dtypes.bfloat16))
    # weight OIHW -> [ci, (kh kw co)] so each lhsT tile is a contiguous slice
    w_host = np.ascontiguousarray(
        np.asarray(weight, dtype=np.float32)
        .transpose(1, 2, 3, 0).reshape(C_IN, 9 * C_OUT)
        .astype(ml_dtypes.bfloat16)
    )
    # bias[co] -> [co % 128, co // 128]
    b_host = np.ascontiguousarray(
        np.asarray(bias, dtype=np.float32).reshape(N_CT, 128).T)
    return xp, w_host, b_host


def kernel(x, weight, bias, _trace=False):
    xp, w_host, b_host = _host_prep(x, weight, bias)
    nc = build_nc()
    in_maps = [
        {"x": xp[i * N_PER_CORE:(i + 1) * N_PER_CORE], "w": w_host, "b": b_host}
        for i in range(N_CORES)
    ]
    res = run_bass_kernel_spmd(nc, in_maps, core_ids=list(range(N_CORES)), trace=_trace)
    out = np.concatenate(
        [res.results[i]["out"].reshape(N_PER_CORE, C_OUT, H, W) for i in range(N_CORES)],
        axis=0,
    )
    if _trace:
        return out, res
    return out
